# revision 31
# baseline (speedup 1.0000x reference)
"""LRU (Linear Recurrent Unit) block kernel for Trainium2, 8 NeuronCores.

Math (per batch element, see reference):
    lam  = exp(-exp(nu_log)) * exp(i*exp(theta_log))          (S,) complex
    Bn   = (B_re + i B_im) * exp(gamma_log)[:, None]          (S, D)
    Bu_t = Bn @ x_t                                           complex
    s_t  = lam * s_{t-1} + Bu_t                               diagonal complex scan
    z_t  = Re(C s_t) + D x_t
    out  = W_proj @ gelu(W_fc @ z + b_fc) + b_proj + x        (MLP + residual)

Device strategy (data-parallel over batch, 2 sequences/core):
  - Everything runs transposed: features on SBUF partitions, tokens on the
    free axis. x is pre-transposed on the host.
  - Complex scan via the modulus-phase decomposition: with lam = r*e^{i*th},
    v_tau = e^{-i*th*tau} s_tau obeys v_tau = r v_{tau-1} + e^{-i*th*tau} Bu_tau
    — TWO REAL first-order recurrences (DVE tensor_tensor_scan).
  - The twiddle (cos/sin modulation) runs in fp16 on the DVE in 2x mode;
    every element-wise op covers both 128-state halves at once via 4D APs
    (half the instruction count). Ops must NOT alias out with an input —
    in-place tensor_tensor silently drops to 1x mode. GpSimd is left idle
    on purpose: its software tensor ops contend for SBUF ports and slow
    the DVE ~2.5x while active.
  - PSUM is managed as one FIFO pool of four 2-bank (4KB) tiles; every
    scalar-engine PSUM drain (Bu copy, z copy, gelu, output bias) covers
    two banks per instruction, halving the scalar op count.
  - Phase pipeline per chunk: A(k+1) Bu-matmuls | C(k) MLP matmuls | B(k+1)
    twiddle+scan on the DVE, with the untwiddle split in halves so phase C
    can start on the first half-chunk early.
"""

import numpy as np

import concourse.bass as bass
import concourse.mybir as mybir
import concourse.tile as tile
from concourse.vector_clock import ScopedClock
from concourse.bass_utils import run_bass_kernel_spmd

Alu = mybir.AluOpType
F32 = mybir.dt.float32
F16 = mybir.dt.float16
ACTF = mybir.ActivationFunctionType
GELU_FUNC = ACTF.Gelu  # overridable for CoreSim (no Gelu in the interpreter)

BATCH, SEQLEN, DM, DS, DF = 16, 8192, 256, 256, 1024
NCORES = 8
NSEQ = BATCH // NCORES          # sequences per core
PC = 256                        # positions per PSUM sub-chunk (per sequence)
SCMAX = 1024                    # max positions per super-chunk
CHUNKS = (1024, 1024, 1024, 1024, 1024, 1024, 1024, 1024)
assert sum(CHUNKS) == SEQLEN

# ---- fp32 consts blob layout (columns of [128, NCOL]) ----------------------
RT0 = 0                         # scan decay r, [st][tau]: 2*SCMAX fp32 cols
ROT0 = RT0 + 2 * SCMAX          # carry rotation cos per (chunk, st), then sin
NCHUNK = len(CHUNKS)
BFC0 = ROT0 + 4 * NCHUNK        # fc1 bias per f-tile (8)
BPJ0 = BFC0 + 8                 # proj bias per o-tile (2)
NCOL = BPJ0 + 2

# ---- fp16 consts blob layout (columns of [128, NCOLH]) ---------------------
# 21 fp16 weight tiles: cr 4, cm 4, bnre 4, bnim 4, dT 4, identity 1
def _hi_cr(st, ot): return st * 2 + ot      # C_re^T tiles
def _hi_cm(st, ot): return 4 + st * 2 + ot  # -C_im^T tiles
def _hi_bnre(kt, st): return 8 + kt * 2 + st
def _hi_bnim(kt, st): return 12 + kt * 2 + st
def _hi_dT(kt, ot):   return 16 + kt * 2 + ot
_HI_IDENT = 20
def _hi_wfc(kt, ft):  return 21 + kt * 8 + ft
def _hi_wpj(ft, ot):  return 37 + ft * 2 + ot
CH0 = 53 * 128                  # cos table [st][tau]: 2*SCMAX cols
SH0 = CH0 + 2 * SCMAX           # sin table
NCOLH = SH0 + 2 * SCMAX


# --- tile-exit drain workaround: walrus in this container caps the sync-wait
# slots on a TPB_CTRL Drain; split the waits onto follow-up SP nops. ---------
def _patched_drain_and_barrier(self, tick_clock, wait_clock):
    nc = self.nc
    drain_inst = nc.sync.drain()
    wait_clock.add_sem_waits(
        drain_inst.ins, ScopedClock({None: tick_clock.global_clock})
    )
    si = drain_inst.ins.sync_info
    if si is not None and si.on_wait and len(si.on_wait) > 1:
        waits = list(si.on_wait)
        drain_inst.ins.sync_info = mybir.SyncInfo(
            on_wait=[waits[0]], on_update=list(si.on_update or [])
        )
        for w in waits[1:]:
            nop = nc.sync.nop(hint="drain_wait_split", nofuse=True)
            nop.ins.sync_info = mybir.SyncInfo(on_wait=[w], on_update=[])
    nc.all_engine_barrier()
    assert self.sems is not None
    popped = nc._tile_sem_poison_stack.pop()
    assert popped is self._sem_poison
    nc.clear_and_free_semaphores(list(self.sems.allocated().values()))
    nc.all_engine_barrier()


tile.TileContext._drain_and_barrier = _patched_drain_and_barrier


# --- universal sync-wait splitter: this walrus rejects >1 wait on several
# instruction structs (S3_LW matmul, TPB_CTRL drain, ...). Rewrite the
# serialized BIR so every instruction carries at most one wait; extras move
# to injected same-engine NoOps placed immediately before it. ----------------
def _split_sync_waits(bir: bytes) -> bytes:
    import json as _json

    m = _json.loads(bir)
    ctr = 0
    for f in m.get("functions", []):
        for bb in f.get("blocks", []):
            insts = bb.get("instructions")
            if not insts:
                continue
            out = []
            for inst in insts:
                si = inst.get("sync_info")
                ow = (si or {}).get("on_wait") or []
                if len(ow) > 1:
                    for wdesc in ow[:-1]:
                        ctr += 1
                        out.append({
                            "engine": inst["engine"],
                            "ins": [],
                            "outs": [],
                            "name": f"I-wsplit{ctr}",
                            "opcode": "NoOp",
                            "sync_info": {"on_update": [], "on_wait": [wdesc]},
                            "text_hint": "wait_split",
                        })
                    si["on_wait"] = [ow[-1]]
                out.append(inst)
            bb["instructions"] = out
    return _json.dumps(m).encode()


_orig_to_json_bytes = bass.Bass.to_json_bytes


def _to_json_bytes_split(self):
    return _split_sync_waits(_orig_to_json_bytes(self))


bass.Bass.to_json_bytes = _to_json_bytes_split


def _enable_axon_ntff_profiling():
    """Best-effort: register the axon NTFF profile hook (the image's antenv
    lacks axon_hooks; the backing ctypes impl ships in trn_agent_boot) and
    neuter the S3 artifact upload the trace path would attempt."""
    try:
        import sys, types
        try:
            import antenv.axon_hooks  # noqa: F401
        except ImportError:
            mod = types.ModuleType("antenv.axon_hooks")
            mod._hook = None

            def set_axon_ntff_profile_hook(h):
                mod._hook = h

            def get_axon_ntff_profile_hook():
                return mod._hook

            mod.set_axon_ntff_profile_hook = set_axon_ntff_profile_hook
            mod.get_axon_ntff_profile_hook = get_axon_ntff_profile_hook
            sys.modules["antenv.axon_hooks"] = mod
            import antenv
            antenv.axon_hooks = mod
        import antenv.axon_hooks as ah
        if ah.get_axon_ntff_profile_hook() is None:
            from trn_agent_boot.trn_boot import _ntff_profile_via_ctypes
            ah.set_axon_ntff_profile_hook(
                _ntff_profile_via_ctypes("/opt/axon/libaxon_pjrt.so")
            )
        import concourse.bass_utils as bu
        bu.upload_artifacts = lambda tmpdir: ""
    except Exception:
        pass


import os as _os
if _os.environ.get("BASS_TRACE"):
    _enable_axon_ntff_profiling()


def build_nc():
    """Per-core Bass module. Token layout: [nseq, seqlen] flattened."""
    ntok = NSEQ * SEQLEN

    nc = bass.Bass()
    xT = nc.declare_dram_parameter("xT", [2, 128, ntok], F16, isOutput=False)
    consts = nc.declare_dram_parameter("consts", [128, NCOL], F32, isOutput=False)
    constsh = nc.declare_dram_parameter("constsh", [128, NCOLH], F16, isOutput=False)
    outT = nc.declare_dram_parameter("outT", [2, 128, ntok], F32, isOutput=True)

    xTv = [xT[kt].rearrange("p (b l) -> p b l", b=NSEQ) for kt in range(2)]
    outTv = [outT[ot].rearrange("p (b l) -> p b l", b=NSEQ) for ot in range(2)]

    from contextlib import ExitStack
    with tile.TileContext(nc) as tc, ExitStack() as ctx:
        singles = ctx.enter_context(tc.tile_pool(name="singles", bufs=1))
        xts = ctx.enter_context(tc.tile_pool(name="xts", bufs=3))
        bus = ctx.enter_context(tc.tile_pool(name="bus", bufs=2))
        uts = ctx.enter_context(tc.tile_pool(name="uts", bufs=1))
        ss = ctx.enter_context(tc.tile_pool(name="ss", bufs=2))
        zs_p = ctx.enter_context(tc.tile_pool(name="zs", bufs=2))
        hs_p = ctx.enter_context(tc.tile_pool(name="hs", bufs=2))
        obs = ctx.enter_context(tc.tile_pool(name="obs", bufs=2))
        tmps = ctx.enter_context(tc.tile_pool(name="tmps", bufs=2))
        cartmps = ctx.enter_context(tc.tile_pool(name="cartmps", bufs=2))
        carries = ctx.enter_context(tc.tile_pool(name="carries", bufs=2))
        # paired-bank PSUM tiles: [128, 2(sub), NSEQ, PC] f32 = 4KB/part
        ps = ctx.enter_context(tc.tile_pool(name="ps", bufs=4, space="PSUM"))

        # consts DMA split by first consumer: Bn weights gate phase A(0),
        # cb (scan decay) + trig tables gate B(0), MLP weights gate C(0).
        cb = singles.tile([128, NCOL], F32, tag="consts")
        ch = singles.tile([128, NCOLH], F16, tag="constsh")
        nc.sync.dma_start(out=ch[:, 8 * 128:16 * 128],
                          in_=constsh[:, 8 * 128:16 * 128])
        nc.sync.dma_start(out=cb[:], in_=consts[:])
        nc.sync.dma_start(out=ch[:, CH0:], in_=constsh[:, CH0:])
        nc.sync.dma_start(out=ch[:, 0:8 * 128], in_=constsh[:, 0:8 * 128])
        nc.sync.dma_start(out=ch[:, 16 * 128:CH0],
                          in_=constsh[:, 16 * 128:CH0])

        def wh(i):  # fp16 weight tile i
            return ch[:, i * 128:(i + 1) * 128]

        def tab4(base, a, bnd):  # fp16 table [128, 2st, nseq, W] bcast on seq
            return ch[:, base: base + 2 * SCMAX] \
                .rearrange("p (s t) -> p s t", s=2)[:, :, None, a:bnd] \
                .to_broadcast([128, 2, NSEQ, bnd - a])

        # carry state [128, st, plane, b], fp32, zero-init
        carry = carries.tile([128, 2, 2, NSEQ], F32, tag="carry")
        nc.vector.memset(carry[:], 0.0)

        def phase_A(ci, lo, L):
            """Load x chunk, compute Bu into SBUF fp16 (via shared PSUM)."""
            subs = L // PC
            xt = []
            for kt in range(2):
                t = xts.tile([128, NSEQ, SCMAX], F16, tag=f"xt{kt}")
                nc.sync.dma_start(out=t[:, :, :L], in_=xTv[kt][:, :, lo:lo + L])
                xt.append(t)
            bu_re = bus.tile([128, 2, NSEQ, SCMAX], F16, tag="bure",
                             name="bure")
            bu_im = bus.tile([128, 2, NSEQ, SCMAX], F16, tag="buim",
                             name="buim")
            for s0 in range(0, subs, 2):
                g = min(2, subs - s0)
                for st in range(2):
                    for pl, bt in ((0, bu_re), (1, bu_im)):
                        psb = ps.tile([128, 2, NSEQ, PC], F32, tag="ps")
                        for gi in range(g):
                            o0 = (s0 + gi) * PC
                            for kt in range(2):
                                wi = (_hi_bnre(kt, st) if pl == 0
                                      else _hi_bnim(kt, st))
                                nc.tensor.matmul(
                                    psb[:, gi], wh(wi),
                                    xt[kt][:, :, o0:o0 + PC],
                                    start=(kt == 0), stop=(kt == 1))
                        bslice = bt[:, st, :, s0 * PC:(s0 + g) * PC] \
                            .rearrange("p b (g f) -> p g b f", g=g)
                        nc.scalar.activation(bslice, psb[:, :g], ACTF.Copy)
            return xt, (bu_re, bu_im)

        def phase_B(ci, L, bu_sb):
            """Twiddle -> scan -> untwiddle, all on the DVE. Every
            element-wise op covers BOTH state halves (4D APs) to halve the
            instruction count; op order minimizes the latency until phase
            C's first-half s tiles are available."""
            nonlocal carry
            bu_re, bu_im = bu_sb
            carry_new = carries.tile([128, 2, 2, NSEQ], F32, tag="carry")

            ur_t = uts.tile([128, 2, NSEQ, SCMAX], F16, tag="utre")
            ui_t = uts.tile([128, 2, NSEQ, SCMAX], F16, tag="utim")
            sr_t = ss.tile([128, 2, NSEQ, SCMAX], F16, tag="sre")
            si_t = ss.tile([128, 2, NSEQ, SCMAX], F16, tag="sim")

            def fwd(a, bnd):
                # forward twiddle (rotating frame): u = e^{-i*th*tau} * Bu
                cosb, sinb = tab4(CH0, a, bnd), tab4(SH0, a, bnd)
                W = bnd - a
                ur, ui = ur_t[:, :, :, a:bnd], ui_t[:, :, :, a:bnd]
                bre = bu_re[:, :, :, a:bnd]
                bim = bu_im[:, :, :, a:bnd]
                t1 = tmps.tile([128, 2, NSEQ, SCMAX], F16, tag="twtmp")
                t2 = tmps.tile([128, 2, NSEQ, SCMAX], F16, tag="twtmp")
                nc.vector.tensor_tensor(t1[:, :, :, :W], cosb, bre, Alu.mult)
                nc.vector.tensor_tensor(t2[:, :, :, :W], sinb, bim, Alu.mult)
                nc.vector.tensor_tensor(ur, t1[:, :, :, :W], t2[:, :, :, :W],
                                        Alu.add)
                t3 = tmps.tile([128, 2, NSEQ, SCMAX], F16, tag="twtmp")
                t4 = tmps.tile([128, 2, NSEQ, SCMAX], F16, tag="twtmp")
                nc.vector.tensor_tensor(t3[:, :, :, :W], cosb, bim, Alu.mult)
                nc.vector.tensor_tensor(t4[:, :, :, :W], sinb, bre, Alu.mult)
                nc.vector.tensor_tensor(ui, t3[:, :, :, :W], t4[:, :, :, :W],
                                        Alu.subtract)

            def scans(st, a, bnd):
                # scans run in place: v overwrites ut; chained at half
                # boundaries via initial = previous half's last column
                rt2 = cb[:, RT0 + st * SCMAX + a: RT0 + st * SCMAX + bnd]
                for pl, t in ((0, ur_t), (1, ui_t)):
                    for b in range(NSEQ):
                        init = (carry[:, st, pl, b:b + 1] if a == 0
                                else t[:, st, b, a - 1:a])
                        nc.vector.tensor_tensor_scan(
                            t[:, st, b, a:bnd], rt2, t[:, st, b, a:bnd],
                            init, Alu.mult, Alu.add)

            def carry_upd():
                # carry for the next chunk: rotate by e^{+i*th*L}
                rotc = cb[:, ROT0 + ci * 2: ROT0 + ci * 2 + 2][:, :, None] \
                    .to_broadcast([128, 2, NSEQ])
                rots = cb[:, ROT0 + 2 * NCHUNK + ci * 2:
                          ROT0 + 2 * NCHUNK + ci * 2 + 2][:, :, None] \
                    .to_broadcast([128, 2, NSEQ])
                vrl = ur_t[:, :, :, L - 1]
                vil = ui_t[:, :, :, L - 1]
                ta = cartmps.tile([128, 2, NSEQ], F32, tag="cartmp")
                tb = cartmps.tile([128, 2, NSEQ], F32, tag="cartmp")
                nc.vector.tensor_tensor(ta[:], rots, vil, Alu.mult)
                nc.vector.tensor_tensor(tb[:], rotc, vrl, Alu.mult)
                nc.vector.tensor_tensor(carry_new[:, :, 0, :], tb[:], ta[:],
                                        Alu.subtract)
                tc = cartmps.tile([128, 2, NSEQ], F32, tag="cartmp")
                td = cartmps.tile([128, 2, NSEQ], F32, tag="cartmp")
                nc.vector.tensor_tensor(tc[:], rots, vrl, Alu.mult)
                nc.vector.tensor_tensor(td[:], rotc, vil, Alu.mult)
                nc.vector.tensor_tensor(carry_new[:, :, 1, :], td[:], tc[:],
                                        Alu.add)

            def untw(a, bnd):
                # untwiddle: s = e^{+i*th*tau} * v  (cols [a, bnd))
                cosb = tab4(CH0, a, bnd)
                sinb = tab4(SH0, a, bnd)
                vr = ur_t[:, :, :, a:bnd]
                vi = ui_t[:, :, :, a:bnd]
                sr = sr_t[:, :, :, a:bnd]
                si_ = si_t[:, :, :, a:bnd]
                u1 = tmps.tile([128, 2, NSEQ, SCMAX], F16, tag="twtmp")
                u2 = tmps.tile([128, 2, NSEQ, SCMAX], F16, tag="twtmp")
                W = bnd - a
                nc.vector.tensor_tensor(u1[:, :, :, :W], cosb, vr, Alu.mult)
                nc.vector.tensor_tensor(u2[:, :, :, :W], sinb, vi, Alu.mult)
                nc.vector.tensor_tensor(sr, u1[:, :, :, :W], u2[:, :, :, :W],
                                        Alu.subtract)
                u3 = tmps.tile([128, 2, NSEQ, SCMAX], F16, tag="twtmp")
                u4 = tmps.tile([128, 2, NSEQ, SCMAX], F16, tag="twtmp")
                nc.vector.tensor_tensor(u3[:, :, :, :W], cosb, vi, Alu.mult)
                nc.vector.tensor_tensor(u4[:, :, :, :W], sinb, vr, Alu.mult)
                nc.vector.tensor_tensor(si_, u3[:, :, :, :W], u4[:, :, :, :W],
                                        Alu.add)

            half = max(PC, (L + PC - 1) // (2 * PC) * PC)  # first-half cols
            fwd(0, half)
            scans(0, 0, half)
            scans(1, 0, half)
            if half < L:
                fwd(half, L)
                untw(0, half)
                scans(0, half, L)
                scans(1, half, L)
                carry_upd()
                untw(half, L)
            else:
                untw(0, half)
                carry_upd()

            carry = carry_new
            return sr_t, si_t

        def phase_C(ci, lo, L, xt, s_re, s_im):
            subs = L // PC
            for s0 in range(0, subs, 2):
                g = min(2, subs - s0)
                q0 = s0 * PC
                glo = lo + q0

                z_sb = []
                for ot in range(2):
                    zp = ps.tile([128, 2, NSEQ, PC], F32, tag="ps")
                    for gi in range(g):
                        o0, o1 = q0 + gi * PC, q0 + (gi + 1) * PC
                        zg = zp[:, gi]
                        nc.tensor.matmul(zg, wh(_hi_dT(0, ot)),
                                         xt[0][:, :, o0:o1],
                                         start=True, stop=False)
                        nc.tensor.matmul(zg, wh(_hi_dT(1, ot)),
                                         xt[1][:, :, o0:o1],
                                         start=False, stop=False)
                        nc.tensor.matmul(zg, wh(_hi_cr(0, ot)),
                                         s_re[:, 0, :, o0:o1],
                                         start=False, stop=False)
                        nc.tensor.matmul(zg, wh(_hi_cm(0, ot)),
                                         s_im[:, 0, :, o0:o1],
                                         start=False, stop=False)
                        nc.tensor.matmul(zg, wh(_hi_cr(1, ot)),
                                         s_re[:, 1, :, o0:o1],
                                         start=False, stop=False)
                        nc.tensor.matmul(zg, wh(_hi_cm(1, ot)),
                                         s_im[:, 1, :, o0:o1],
                                         start=False, stop=True)
                    zt = zs_p.tile([128, 2, NSEQ, PC], F16, tag=f"z{ot}")
                    nc.scalar.activation(zt[:, :g], zp[:, :g], ACTF.Copy)
                    z_sb.append(zt)

                h_sb = []
                for ft in range(8):
                    hp = ps.tile([128, 2, NSEQ, PC], F32, tag="ps")
                    for gi in range(g):
                        nc.tensor.matmul(hp[:, gi], wh(_hi_wfc(0, ft)),
                                         z_sb[0][:, gi], start=True, stop=False)
                        nc.tensor.matmul(hp[:, gi], wh(_hi_wfc(1, ft)),
                                         z_sb[1][:, gi], start=False, stop=True)
                    ht = hs_p.tile([128, 2, NSEQ, PC], F16, tag=f"h{ft}")
                    nc.scalar.activation(
                        ht[:, :g], hp[:, :g], GELU_FUNC,
                        bias=cb[:, BFC0 + ft: BFC0 + ft + 1],
                        scale=1.0)
                    h_sb.append(ht)

                for ot in range(2):
                    qp = ps.tile([128, 2, NSEQ, PC], F32, tag="ps")
                    for gi in range(g):
                        o0, o1 = q0 + gi * PC, q0 + (gi + 1) * PC
                        nc.tensor.matmul(qp[:, gi], wh(_HI_IDENT),
                                         xt[ot][:, :, o0:o1],
                                         start=True, stop=False)
                        for ft in range(8):
                            nc.tensor.matmul(qp[:, gi], wh(_hi_wpj(ft, ot)),
                                             h_sb[ft][:, gi],
                                             start=False, stop=(ft == 7))
                    ob = obs.tile([128, 2, NSEQ, PC], F32, tag=f"ob{ot}")
                    nc.scalar.activation(
                        ob[:, :g], qp[:, :g], ACTF.Identity,
                        bias=cb[:, BPJ0 + ot: BPJ0 + ot + 1],
                        scale=1.0)
                    for gi in range(g):
                        nc.sync.dma_start(
                            out=outTv[ot][:, :, glo + gi * PC:
                                          glo + (gi + 1) * PC],
                            in_=ob[:, gi])

        # software pipeline: A(c) | C(c-1) | B(c), so the PE stream
        # always has ready work while the DVE/GpSimd run phase B.
        prev = None
        lo = 0
        for ci, L in enumerate(CHUNKS):
            xt, bu_sb = phase_A(ci, lo, L)
            if prev is not None:
                phase_C(*prev)
            s_re, s_im = phase_B(ci, L, bu_sb)
            prev = (ci, lo, L, xt, s_re, s_im)
            lo += L
        phase_C(*prev)
    return nc


def pack_consts(nu_log, theta_log, gamma_log, B_re, B_im, C_re, C_im, D,
                W_fc, b_fc, W_proj, b_proj):
    """Assemble the f32 and fp16 consts blobs (tables in float64)."""
    f8 = np.float64
    nu = np.exp(np.asarray(nu_log, f8))
    r = np.exp(-nu)
    theta = np.exp(np.asarray(theta_log, f8))
    gamma = np.exp(np.asarray(gamma_log, f8))
    Bn_re = np.asarray(B_re, f8) * gamma[:, None]
    Bn_im = np.asarray(B_im, f8) * gamma[:, None]
    C_re = np.asarray(C_re, f8)
    C_im = np.asarray(C_im, f8)
    D = np.asarray(D, f8)
    W_fc = np.asarray(W_fc, f8)
    W_proj = np.asarray(W_proj, f8)

    cb = np.zeros((128, NCOL), np.float32)
    ch = np.zeros((128, NCOLH), np.float16)

    def puth(i, m):
        ch[:, i * 128:(i + 1) * 128] = np.asarray(m, np.float16)

    for kt in range(2):
        for st in range(2):
            puth(_hi_bnre(kt, st),
                 Bn_re[st * 128:(st + 1) * 128, kt * 128:(kt + 1) * 128].T)
            puth(_hi_bnim(kt, st),
                 Bn_im[st * 128:(st + 1) * 128, kt * 128:(kt + 1) * 128].T)
    for st in range(2):
        for ot in range(2):
            puth(_hi_cr(st, ot),
                 C_re[ot * 128:(ot + 1) * 128, st * 128:(st + 1) * 128].T)
            puth(_hi_cm(st, ot),
                 -C_im[ot * 128:(ot + 1) * 128, st * 128:(st + 1) * 128].T)
    for kt in range(2):
        for ot in range(2):
            puth(_hi_dT(kt, ot),
                 D[ot * 128:(ot + 1) * 128, kt * 128:(kt + 1) * 128].T)
    for kt in range(2):
        for ft in range(8):
            puth(_hi_wfc(kt, ft),
                 W_fc[kt * 128:(kt + 1) * 128, ft * 128:(ft + 1) * 128])
    for ft in range(8):
        for ot in range(2):
            puth(_hi_wpj(ft, ot),
                 W_proj[ft * 128:(ft + 1) * 128, ot * 128:(ot + 1) * 128])
    puth(_HI_IDENT, np.eye(128))

    tau = np.arange(SCMAX, dtype=f8)
    for st in range(2):
        th = theta[st * 128:(st + 1) * 128]
        ang = th[:, None] * tau[None, :]
        ch[:, CH0 + st * SCMAX: CH0 + (st + 1) * SCMAX] = np.cos(ang)
        ch[:, SH0 + st * SCMAX: SH0 + (st + 1) * SCMAX] = np.sin(ang)
        cb[:, RT0 + st * SCMAX: RT0 + (st + 1) * SCMAX] = \
            r[st * 128:(st + 1) * 128, None]
        for ci, L in enumerate(CHUNKS):
            cb[:, ROT0 + ci * 2 + st] = np.cos(th * L)
            cb[:, ROT0 + 2 * NCHUNK + ci * 2 + st] = np.sin(th * L)
    for ft in range(8):
        cb[:, BFC0 + ft] = np.asarray(b_fc, np.float32)[ft * 128:(ft + 1) * 128]
    for ot in range(2):
        cb[:, BPJ0 + ot] = np.asarray(b_proj, np.float32)[ot * 128:(ot + 1) * 128]
    return cb, ch


_NC_CACHE = {}
LAST_RUN_INFO = {}


def kernel(x, nu_log, theta_log, gamma_log, B_re, B_im, C_re, C_im, D,
           W_fc, b_fc, W_proj, b_proj):
    x = np.asarray(x, np.float32)
    assert x.shape == (BATCH, SEQLEN, DM)

    key = (SEQLEN, NSEQ, PC, CHUNKS)
    if key not in _NC_CACHE:
        _NC_CACHE[key] = build_nc()
    nc = _NC_CACHE[key]

    cb, ch = pack_consts(nu_log, theta_log, gamma_log, B_re, B_im, C_re, C_im,
                         D, W_fc, b_fc, W_proj, b_proj)

    in_maps = []
    for c in range(NCORES):
        xc = x[c * NSEQ:(c + 1) * NSEQ]                      # (nseq, L, D)
        xT = np.ascontiguousarray(
            xc.transpose(2, 0, 1).reshape(2, 128, NSEQ * SEQLEN)
        ).astype(np.float16)
        in_maps.append({"xT": xT, "consts": cb, "constsh": ch})

    res = run_bass_kernel_spmd(nc, in_maps, core_ids=list(range(NCORES)))
    LAST_RUN_INFO.clear()
    LAST_RUN_INFO.update(
        exec_time_ns=res.exec_time_ns,
        mean_exec_time_ns=res.mean_exec_time_ns,
        trace=res.instructions_and_trace[1] if res.instructions_and_trace else None,
        profile_json=res.profile_json,
    )

    out = np.empty((BATCH, SEQLEN, DM), np.float32)
    for c in range(NCORES):
        oT = res.results[c]["outT"]                          # (2, 128, ntok)
        out[c * NSEQ:(c + 1) * NSEQ] = (
            oT.reshape(DM, NSEQ, SEQLEN).transpose(1, 2, 0)
        )
    return out


# revision 35
# speedup vs baseline: 1.0062x; 1.0062x over previous
"""LRU (Linear Recurrent Unit) block kernel for Trainium2, 8 NeuronCores.

Math (per batch element, see reference):
    lam  = exp(-exp(nu_log)) * exp(i*exp(theta_log))          (S,) complex
    Bn   = (B_re + i B_im) * exp(gamma_log)[:, None]          (S, D)
    Bu_t = Bn @ x_t                                           complex
    s_t  = lam * s_{t-1} + Bu_t                               diagonal complex scan
    z_t  = Re(C s_t) + D x_t
    out  = W_proj @ gelu(W_fc @ z + b_fc) + b_proj + x        (MLP + residual)

Device strategy (data-parallel over batch, 2 sequences/core):
  - Everything runs transposed: features on SBUF partitions, tokens on the
    free axis. x is pre-transposed on the host.
  - Complex scan via the modulus-phase decomposition: with lam = r*e^{i*th},
    v_tau = e^{-i*th*tau} s_tau obeys v_tau = r v_{tau-1} + e^{-i*th*tau} Bu_tau
    — TWO REAL first-order recurrences (DVE tensor_tensor_scan).
  - The twiddle (cos/sin modulation) runs in fp16 on the DVE in 2x mode;
    every element-wise op covers both 128-state halves at once via 4D APs
    (half the instruction count). Ops must NOT alias out with an input —
    in-place tensor_tensor silently drops to 1x mode. GpSimd is left idle
    on purpose: its software tensor ops contend for SBUF ports and slow
    the DVE ~2.5x while active.
  - PSUM is managed as one FIFO pool of four 2-bank (4KB) tiles; every
    scalar-engine PSUM drain (Bu copy, z copy, gelu, output bias) covers
    two banks per instruction, halving the scalar op count.
  - Phase pipeline per chunk: A(k+1) Bu-matmuls | C(k) MLP matmuls | B(k+1)
    twiddle+scan on the DVE, with the untwiddle split in halves so phase C
    can start on the first half-chunk early.
"""

import numpy as np

import concourse.bass as bass
import concourse.mybir as mybir
import concourse.tile as tile
from concourse.vector_clock import ScopedClock
from concourse.bass_utils import run_bass_kernel_spmd

Alu = mybir.AluOpType
F32 = mybir.dt.float32
F16 = mybir.dt.float16
ACTF = mybir.ActivationFunctionType
GELU_FUNC = ACTF.Gelu  # overridable for CoreSim (no Gelu in the interpreter)

BATCH, SEQLEN, DM, DS, DF = 16, 8192, 256, 256, 1024
NCORES = 8
NSEQ = BATCH // NCORES          # sequences per core
PC = 256                        # positions per PSUM sub-chunk (per sequence)
SCMAX = 1024                    # max positions per super-chunk
CHUNKS = (1024, 1024, 1024, 1024, 1024, 1024, 1024, 1024)
assert sum(CHUNKS) == SEQLEN

# ---- fp32 consts blob layout (columns of [128, NCOL]) ----------------------
RT0 = 0                         # scan decay r per st: 2 fp32 cols
ROT0 = RT0 + 2                  # carry rotation cos per (chunk, st), then sin
NCHUNK = len(CHUNKS)
BFC0 = ROT0 + 4 * NCHUNK        # fc1 bias per f-tile (8)
BPJ0 = BFC0 + 8                 # proj bias per o-tile (2)
NCOL = BPJ0 + 2

# ---- fp16 consts blob layout (columns of [128, NCOLH]) ---------------------
# 21 fp16 weight tiles: cr 4, cm 4, bnre 4, bnim 4, dT 4, identity 1
def _hi_cr(st, ot): return st * 2 + ot      # C_re^T tiles
def _hi_cm(st, ot): return 4 + st * 2 + ot  # -C_im^T tiles
def _hi_bnre(kt, st): return 8 + kt * 2 + st
def _hi_bnim(kt, st): return 12 + kt * 2 + st
def _hi_dT(kt, ot):   return 16 + kt * 2 + ot
_HI_IDENT = 20
def _hi_wfc(kt, ft):  return 21 + kt * 8 + ft
def _hi_wpj(ft, ot):  return 37 + ft * 2 + ot
CH0 = 53 * 128                  # cos table [st][tau]: 2*SCMAX cols
SH0 = CH0 + 2 * SCMAX           # sin table
NCOLH = SH0 + 2 * SCMAX


# --- tile-exit drain workaround: walrus in this container caps the sync-wait
# slots on a TPB_CTRL Drain; split the waits onto follow-up SP nops. ---------
def _patched_drain_and_barrier(self, tick_clock, wait_clock):
    nc = self.nc
    drain_inst = nc.sync.drain()
    wait_clock.add_sem_waits(
        drain_inst.ins, ScopedClock({None: tick_clock.global_clock})
    )
    si = drain_inst.ins.sync_info
    if si is not None and si.on_wait and len(si.on_wait) > 1:
        waits = list(si.on_wait)
        drain_inst.ins.sync_info = mybir.SyncInfo(
            on_wait=[waits[0]], on_update=list(si.on_update or [])
        )
        for w in waits[1:]:
            nop = nc.sync.nop(hint="drain_wait_split", nofuse=True)
            nop.ins.sync_info = mybir.SyncInfo(on_wait=[w], on_update=[])
    nc.all_engine_barrier()
    assert self.sems is not None
    popped = nc._tile_sem_poison_stack.pop()
    assert popped is self._sem_poison
    nc.clear_and_free_semaphores(list(self.sems.allocated().values()))
    nc.all_engine_barrier()


tile.TileContext._drain_and_barrier = _patched_drain_and_barrier


# --- universal sync-wait splitter: this walrus rejects >1 wait on several
# instruction structs (S3_LW matmul, TPB_CTRL drain, ...). Rewrite the
# serialized BIR so every instruction carries at most one wait; extras move
# to injected same-engine NoOps placed immediately before it. ----------------
def _split_sync_waits(bir: bytes) -> bytes:
    import json as _json

    m = _json.loads(bir)
    ctr = 0
    for f in m.get("functions", []):
        for bb in f.get("blocks", []):
            insts = bb.get("instructions")
            if not insts:
                continue
            out = []
            for inst in insts:
                si = inst.get("sync_info")
                ow = (si or {}).get("on_wait") or []
                if len(ow) > 1:
                    for wdesc in ow[:-1]:
                        ctr += 1
                        out.append({
                            "engine": inst["engine"],
                            "ins": [],
                            "outs": [],
                            "name": f"I-wsplit{ctr}",
                            "opcode": "NoOp",
                            "sync_info": {"on_update": [], "on_wait": [wdesc]},
                            "text_hint": "wait_split",
                        })
                    si["on_wait"] = [ow[-1]]
                out.append(inst)
            bb["instructions"] = out
    return _json.dumps(m).encode()


_orig_to_json_bytes = bass.Bass.to_json_bytes


def _to_json_bytes_split(self):
    return _split_sync_waits(_orig_to_json_bytes(self))


bass.Bass.to_json_bytes = _to_json_bytes_split


def _enable_axon_ntff_profiling():
    """Best-effort: register the axon NTFF profile hook (the image's antenv
    lacks axon_hooks; the backing ctypes impl ships in trn_agent_boot) and
    neuter the S3 artifact upload the trace path would attempt."""
    try:
        import sys, types
        try:
            import antenv.axon_hooks  # noqa: F401
        except ImportError:
            mod = types.ModuleType("antenv.axon_hooks")
            mod._hook = None

            def set_axon_ntff_profile_hook(h):
                mod._hook = h

            def get_axon_ntff_profile_hook():
                return mod._hook

            mod.set_axon_ntff_profile_hook = set_axon_ntff_profile_hook
            mod.get_axon_ntff_profile_hook = get_axon_ntff_profile_hook
            sys.modules["antenv.axon_hooks"] = mod
            import antenv
            antenv.axon_hooks = mod
        import antenv.axon_hooks as ah
        if ah.get_axon_ntff_profile_hook() is None:
            from trn_agent_boot.trn_boot import _ntff_profile_via_ctypes
            ah.set_axon_ntff_profile_hook(
                _ntff_profile_via_ctypes("/opt/axon/libaxon_pjrt.so")
            )
        import concourse.bass_utils as bu
        bu.upload_artifacts = lambda tmpdir: ""
    except Exception:
        pass


import os as _os
if _os.environ.get("BASS_TRACE"):
    _enable_axon_ntff_profiling()


def build_nc():
    """Per-core Bass module. Token layout: [nseq, seqlen] flattened."""
    ntok = NSEQ * SEQLEN

    nc = bass.Bass()
    xT = nc.declare_dram_parameter("xT", [2, 128, ntok], F16, isOutput=False)
    consts = nc.declare_dram_parameter("consts", [128, NCOL], F32, isOutput=False)
    constsh = nc.declare_dram_parameter("constsh", [128, NCOLH], F16, isOutput=False)
    outT = nc.declare_dram_parameter("outT", [2, 128, ntok], F32, isOutput=True)

    xTv = [xT[kt].rearrange("p (b l) -> p b l", b=NSEQ) for kt in range(2)]
    outTv = [outT[ot].rearrange("p (b l) -> p b l", b=NSEQ) for ot in range(2)]

    from contextlib import ExitStack
    with tile.TileContext(nc) as tc, ExitStack() as ctx:
        singles = ctx.enter_context(tc.tile_pool(name="singles", bufs=1))
        xts = ctx.enter_context(tc.tile_pool(name="xts", bufs=3))
        bus = ctx.enter_context(tc.tile_pool(name="bus", bufs=2))
        uts = ctx.enter_context(tc.tile_pool(name="uts", bufs=1))
        ss = ctx.enter_context(tc.tile_pool(name="ss", bufs=2))
        zs_p = ctx.enter_context(tc.tile_pool(name="zs", bufs=2))
        hs_p = ctx.enter_context(tc.tile_pool(name="hs", bufs=2))
        obs = ctx.enter_context(tc.tile_pool(name="obs", bufs=2))
        tmps = ctx.enter_context(tc.tile_pool(name="tmps", bufs=2))
        cartmps = ctx.enter_context(tc.tile_pool(name="cartmps", bufs=2))
        carries = ctx.enter_context(tc.tile_pool(name="carries", bufs=2))
        # paired-bank PSUM tiles: [128, 2(sub), NSEQ, PC] f32 = 4KB/part
        ps = ctx.enter_context(tc.tile_pool(name="ps", bufs=4, space="PSUM"))

        # consts DMA split by first consumer: Bn weights gate phase A(0),
        # cb (scan decay) + trig tables gate B(0), MLP weights gate C(0).
        cb = singles.tile([128, NCOL], F32, tag="consts")
        ch = singles.tile([128, NCOLH], F16, tag="constsh")
        nc.sync.dma_start(out=ch[:, 8 * 128:16 * 128],
                          in_=constsh[:, 8 * 128:16 * 128])
        nc.sync.dma_start(out=cb[:], in_=consts[:])
        nc.sync.dma_start(out=ch[:, CH0:], in_=constsh[:, CH0:])
        nc.sync.dma_start(out=ch[:, 0:8 * 128], in_=constsh[:, 0:8 * 128])
        nc.sync.dma_start(out=ch[:, 16 * 128:CH0],
                          in_=constsh[:, 16 * 128:CH0])

        def wh(i):  # fp16 weight tile i
            return ch[:, i * 128:(i + 1) * 128]

        def tab4(base, a, bnd):  # fp16 table [128, 2st, nseq, W] bcast on seq
            return ch[:, base: base + 2 * SCMAX] \
                .rearrange("p (s t) -> p s t", s=2)[:, :, None, a:bnd] \
                .to_broadcast([128, 2, NSEQ, bnd - a])

        # carry state [128, st, plane, b], fp32, zero-init
        carry = carries.tile([128, 2, 2, NSEQ], F32, tag="carry")
        nc.vector.memset(carry[:], 0.0)

        def phase_A(ci, lo, L):
            """Load x chunk, compute Bu into SBUF fp16 (via shared PSUM)."""
            subs = L // PC
            xt = []
            for kt in range(2):
                t = xts.tile([128, NSEQ, SCMAX], F16, tag=f"xt{kt}")
                nc.sync.dma_start(out=t[:, :, :L], in_=xTv[kt][:, :, lo:lo + L])
                xt.append(t)
            bu_re = bus.tile([128, 2, NSEQ, SCMAX], F16, tag="bure",
                             name="bure")
            bu_im = bus.tile([128, 2, NSEQ, SCMAX], F16, tag="buim",
                             name="buim")
            for s0 in range(0, subs, 2):
                g = min(2, subs - s0)
                for pl, bt in ((0, bu_re), (1, bu_im)):
                    for st in range(2):
                        psb = ps.tile([128, 2, NSEQ, PC], F32, tag="ps")
                        for gi in range(g):
                            o0 = (s0 + gi) * PC
                            for kt in range(2):
                                wi = (_hi_bnre(kt, st) if pl == 0
                                      else _hi_bnim(kt, st))
                                nc.tensor.matmul(
                                    psb[:, gi], wh(wi),
                                    xt[kt][:, :, o0:o0 + PC],
                                    start=(kt == 0), stop=(kt == 1))
                        bslice = bt[:, st, :, s0 * PC:(s0 + g) * PC] \
                            .rearrange("p b (g f) -> p g b f", g=g)
                        nc.scalar.activation(bslice, psb[:, :g], ACTF.Copy)
            return xt, (bu_re, bu_im)

        def phase_B(ci, L, bu_sb):
            """Twiddle -> scan -> untwiddle, all on the DVE. Every
            element-wise op covers BOTH state halves (4D APs) to halve the
            instruction count; op order minimizes the latency until phase
            C's first-half s tiles are available."""
            nonlocal carry
            bu_re, bu_im = bu_sb
            carry_new = carries.tile([128, 2, 2, NSEQ], F32, tag="carry")

            ur_t = uts.tile([128, 2, NSEQ, SCMAX], F16, tag="utre")
            ui_t = uts.tile([128, 2, NSEQ, SCMAX], F16, tag="utim")
            sr_t = ss.tile([128, 2, NSEQ, SCMAX], F16, tag="sre")
            si_t = ss.tile([128, 2, NSEQ, SCMAX], F16, tag="sim")

            def fwd(a, bnd):
                # forward twiddle (rotating frame): u = e^{-i*th*tau} * Bu
                cosb, sinb = tab4(CH0, a, bnd), tab4(SH0, a, bnd)
                W = bnd - a
                ur, ui = ur_t[:, :, :, a:bnd], ui_t[:, :, :, a:bnd]
                bre = bu_re[:, :, :, a:bnd]
                bim = bu_im[:, :, :, a:bnd]
                t1 = tmps.tile([128, 2, NSEQ, SCMAX], F16, tag="twtmp")
                t2 = tmps.tile([128, 2, NSEQ, SCMAX], F16, tag="twtmp")
                nc.vector.tensor_tensor(t1[:, :, :, :W], cosb, bre, Alu.mult)
                nc.vector.tensor_tensor(t2[:, :, :, :W], sinb, bim, Alu.mult)
                nc.vector.tensor_tensor(ur, t1[:, :, :, :W], t2[:, :, :, :W],
                                        Alu.add)
                t3 = tmps.tile([128, 2, NSEQ, SCMAX], F16, tag="twtmp")
                t4 = tmps.tile([128, 2, NSEQ, SCMAX], F16, tag="twtmp")
                nc.vector.tensor_tensor(t3[:, :, :, :W], cosb, bim, Alu.mult)
                nc.vector.tensor_tensor(t4[:, :, :, :W], sinb, bre, Alu.mult)
                nc.vector.tensor_tensor(ui, t3[:, :, :, :W], t4[:, :, :, :W],
                                        Alu.subtract)

            def scans(st, a, bnd):
                # scans run in place: v overwrites ut; chained at half
                # boundaries via initial = previous half's last column.
                # decay r is constant along tau: stride-0 broadcast column.
                rt2 = cb[:, RT0 + st: RT0 + st + 1].to_broadcast(
                    [128, bnd - a])
                for pl, t in ((0, ur_t), (1, ui_t)):
                    for b in range(NSEQ):
                        init = (carry[:, st, pl, b:b + 1] if a == 0
                                else t[:, st, b, a - 1:a])
                        nc.vector.tensor_tensor_scan(
                            t[:, st, b, a:bnd], rt2, t[:, st, b, a:bnd],
                            init, Alu.mult, Alu.add)

            def carry_upd():
                # carry for the next chunk: rotate by e^{+i*th*L}
                rotc = cb[:, ROT0 + ci * 2: ROT0 + ci * 2 + 2][:, :, None] \
                    .to_broadcast([128, 2, NSEQ])
                rots = cb[:, ROT0 + 2 * NCHUNK + ci * 2:
                          ROT0 + 2 * NCHUNK + ci * 2 + 2][:, :, None] \
                    .to_broadcast([128, 2, NSEQ])
                vrl = ur_t[:, :, :, L - 1]
                vil = ui_t[:, :, :, L - 1]
                ta = cartmps.tile([128, 2, NSEQ], F32, tag="cartmp")
                tb = cartmps.tile([128, 2, NSEQ], F32, tag="cartmp")
                nc.vector.tensor_tensor(ta[:], rots, vil, Alu.mult)
                nc.vector.tensor_tensor(tb[:], rotc, vrl, Alu.mult)
                nc.vector.tensor_tensor(carry_new[:, :, 0, :], tb[:], ta[:],
                                        Alu.subtract)
                tc = cartmps.tile([128, 2, NSEQ], F32, tag="cartmp")
                td = cartmps.tile([128, 2, NSEQ], F32, tag="cartmp")
                nc.vector.tensor_tensor(tc[:], rots, vrl, Alu.mult)
                nc.vector.tensor_tensor(td[:], rotc, vil, Alu.mult)
                nc.vector.tensor_tensor(carry_new[:, :, 1, :], td[:], tc[:],
                                        Alu.add)

            def untw(a, bnd):
                # untwiddle: s = e^{+i*th*tau} * v  (cols [a, bnd))
                cosb = tab4(CH0, a, bnd)
                sinb = tab4(SH0, a, bnd)
                vr = ur_t[:, :, :, a:bnd]
                vi = ui_t[:, :, :, a:bnd]
                sr = sr_t[:, :, :, a:bnd]
                si_ = si_t[:, :, :, a:bnd]
                u1 = tmps.tile([128, 2, NSEQ, SCMAX], F16, tag="twtmp")
                u2 = tmps.tile([128, 2, NSEQ, SCMAX], F16, tag="twtmp")
                W = bnd - a
                nc.vector.tensor_tensor(u1[:, :, :, :W], cosb, vr, Alu.mult)
                nc.vector.tensor_tensor(u2[:, :, :, :W], sinb, vi, Alu.mult)
                nc.vector.tensor_tensor(sr, u1[:, :, :, :W], u2[:, :, :, :W],
                                        Alu.subtract)
                u3 = tmps.tile([128, 2, NSEQ, SCMAX], F16, tag="twtmp")
                u4 = tmps.tile([128, 2, NSEQ, SCMAX], F16, tag="twtmp")
                nc.vector.tensor_tensor(u3[:, :, :, :W], cosb, vi, Alu.mult)
                nc.vector.tensor_tensor(u4[:, :, :, :W], sinb, vr, Alu.mult)
                nc.vector.tensor_tensor(si_, u3[:, :, :, :W], u4[:, :, :, :W],
                                        Alu.add)

            half = max(PC, (L + PC - 1) // (2 * PC) * PC)  # first-half cols
            fwd(0, half)
            scans(0, 0, half)
            scans(1, 0, half)
            if half < L:
                fwd(half, L)
                untw(0, half)
                scans(0, half, L)
                scans(1, half, L)
                carry_upd()
                untw(half, L)
            else:
                untw(0, half)
                carry_upd()

            carry = carry_new
            return sr_t, si_t

        def phase_C(ci, lo, L, xt, s_re, s_im):
            subs = L // PC
            for s0 in range(0, subs, 2):
                g = min(2, subs - s0)
                q0 = s0 * PC
                glo = lo + q0

                z_sb = []
                for ot in range(2):
                    zp = ps.tile([128, 2, NSEQ, PC], F32, tag="ps")
                    for gi in range(g):
                        o0, o1 = q0 + gi * PC, q0 + (gi + 1) * PC
                        zg = zp[:, gi]
                        nc.tensor.matmul(zg, wh(_hi_dT(0, ot)),
                                         xt[0][:, :, o0:o1],
                                         start=True, stop=False)
                        nc.tensor.matmul(zg, wh(_hi_dT(1, ot)),
                                         xt[1][:, :, o0:o1],
                                         start=False, stop=False)
                        nc.tensor.matmul(zg, wh(_hi_cr(0, ot)),
                                         s_re[:, 0, :, o0:o1],
                                         start=False, stop=False)
                        nc.tensor.matmul(zg, wh(_hi_cm(0, ot)),
                                         s_im[:, 0, :, o0:o1],
                                         start=False, stop=False)
                        nc.tensor.matmul(zg, wh(_hi_cr(1, ot)),
                                         s_re[:, 1, :, o0:o1],
                                         start=False, stop=False)
                        nc.tensor.matmul(zg, wh(_hi_cm(1, ot)),
                                         s_im[:, 1, :, o0:o1],
                                         start=False, stop=True)
                    zt = zs_p.tile([128, 2, NSEQ, PC], F16, tag=f"z{ot}")
                    nc.scalar.activation(zt[:, :g], zp[:, :g], ACTF.Copy)
                    z_sb.append(zt)

                h_sb = []
                for ft in range(8):
                    hp = ps.tile([128, 2, NSEQ, PC], F32, tag="ps")
                    for gi in range(g):
                        nc.tensor.matmul(hp[:, gi], wh(_hi_wfc(0, ft)),
                                         z_sb[0][:, gi], start=True, stop=False)
                        nc.tensor.matmul(hp[:, gi], wh(_hi_wfc(1, ft)),
                                         z_sb[1][:, gi], start=False, stop=True)
                    ht = hs_p.tile([128, 2, NSEQ, PC], F16, tag=f"h{ft}")
                    nc.scalar.activation(
                        ht[:, :g], hp[:, :g], GELU_FUNC,
                        bias=cb[:, BFC0 + ft: BFC0 + ft + 1],
                        scale=1.0)
                    h_sb.append(ht)

                for ot in range(2):
                    qp = ps.tile([128, 2, NSEQ, PC], F32, tag="ps")
                    for gi in range(g):
                        o0, o1 = q0 + gi * PC, q0 + (gi + 1) * PC
                        nc.tensor.matmul(qp[:, gi], wh(_HI_IDENT),
                                         xt[ot][:, :, o0:o1],
                                         start=True, stop=False)
                        for ft in range(8):
                            nc.tensor.matmul(qp[:, gi], wh(_hi_wpj(ft, ot)),
                                             h_sb[ft][:, gi],
                                             start=False, stop=(ft == 7))
                    ob = obs.tile([128, 2, NSEQ, PC], F32, tag=f"ob{ot}")
                    nc.scalar.activation(
                        ob[:, :g], qp[:, :g], ACTF.Identity,
                        bias=cb[:, BPJ0 + ot: BPJ0 + ot + 1],
                        scale=1.0)
                    for gi in range(g):
                        nc.sync.dma_start(
                            out=outTv[ot][:, :, glo + gi * PC:
                                          glo + (gi + 1) * PC],
                            in_=ob[:, gi])

        # software pipeline: A(c) | C(c-1) | B(c), so the PE stream
        # always has ready work while the DVE/GpSimd run phase B.
        prev = None
        lo = 0
        for ci, L in enumerate(CHUNKS):
            xt, bu_sb = phase_A(ci, lo, L)
            if prev is not None:
                phase_C(*prev)
            s_re, s_im = phase_B(ci, L, bu_sb)
            prev = (ci, lo, L, xt, s_re, s_im)
            lo += L
        phase_C(*prev)
    return nc


def pack_consts(nu_log, theta_log, gamma_log, B_re, B_im, C_re, C_im, D,
                W_fc, b_fc, W_proj, b_proj):
    """Assemble the f32 and fp16 consts blobs (tables in float64)."""
    f8 = np.float64
    nu = np.exp(np.asarray(nu_log, f8))
    r = np.exp(-nu)
    theta = np.exp(np.asarray(theta_log, f8))
    gamma = np.exp(np.asarray(gamma_log, f8))
    Bn_re = np.asarray(B_re, f8) * gamma[:, None]
    Bn_im = np.asarray(B_im, f8) * gamma[:, None]
    C_re = np.asarray(C_re, f8)
    C_im = np.asarray(C_im, f8)
    D = np.asarray(D, f8)
    W_fc = np.asarray(W_fc, f8)
    W_proj = np.asarray(W_proj, f8)

    cb = np.zeros((128, NCOL), np.float32)
    ch = np.zeros((128, NCOLH), np.float16)

    def puth(i, m):
        ch[:, i * 128:(i + 1) * 128] = np.asarray(m, np.float16)

    for kt in range(2):
        for st in range(2):
            puth(_hi_bnre(kt, st),
                 Bn_re[st * 128:(st + 1) * 128, kt * 128:(kt + 1) * 128].T)
            puth(_hi_bnim(kt, st),
                 Bn_im[st * 128:(st + 1) * 128, kt * 128:(kt + 1) * 128].T)
    for st in range(2):
        for ot in range(2):
            puth(_hi_cr(st, ot),
                 C_re[ot * 128:(ot + 1) * 128, st * 128:(st + 1) * 128].T)
            puth(_hi_cm(st, ot),
                 -C_im[ot * 128:(ot + 1) * 128, st * 128:(st + 1) * 128].T)
    for kt in range(2):
        for ot in range(2):
            puth(_hi_dT(kt, ot),
                 D[ot * 128:(ot + 1) * 128, kt * 128:(kt + 1) * 128].T)
    for kt in range(2):
        for ft in range(8):
            puth(_hi_wfc(kt, ft),
                 W_fc[kt * 128:(kt + 1) * 128, ft * 128:(ft + 1) * 128])
    for ft in range(8):
        for ot in range(2):
            puth(_hi_wpj(ft, ot),
                 W_proj[ft * 128:(ft + 1) * 128, ot * 128:(ot + 1) * 128])
    puth(_HI_IDENT, np.eye(128))

    tau = np.arange(SCMAX, dtype=f8)
    for st in range(2):
        th = theta[st * 128:(st + 1) * 128]
        ang = th[:, None] * tau[None, :]
        ch[:, CH0 + st * SCMAX: CH0 + (st + 1) * SCMAX] = np.cos(ang)
        ch[:, SH0 + st * SCMAX: SH0 + (st + 1) * SCMAX] = np.sin(ang)
        cb[:, RT0 + st] = r[st * 128:(st + 1) * 128]
        for ci, L in enumerate(CHUNKS):
            cb[:, ROT0 + ci * 2 + st] = np.cos(th * L)
            cb[:, ROT0 + 2 * NCHUNK + ci * 2 + st] = np.sin(th * L)
    for ft in range(8):
        cb[:, BFC0 + ft] = np.asarray(b_fc, np.float32)[ft * 128:(ft + 1) * 128]
    for ot in range(2):
        cb[:, BPJ0 + ot] = np.asarray(b_proj, np.float32)[ot * 128:(ot + 1) * 128]
    return cb, ch


_NC_CACHE = {}
LAST_RUN_INFO = {}


def kernel(x, nu_log, theta_log, gamma_log, B_re, B_im, C_re, C_im, D,
           W_fc, b_fc, W_proj, b_proj):
    x = np.asarray(x, np.float32)
    assert x.shape == (BATCH, SEQLEN, DM)

    key = (SEQLEN, NSEQ, PC, CHUNKS)
    if key not in _NC_CACHE:
        _NC_CACHE[key] = build_nc()
    nc = _NC_CACHE[key]

    cb, ch = pack_consts(nu_log, theta_log, gamma_log, B_re, B_im, C_re, C_im,
                         D, W_fc, b_fc, W_proj, b_proj)

    in_maps = []
    for c in range(NCORES):
        xc = x[c * NSEQ:(c + 1) * NSEQ]                      # (nseq, L, D)
        xT = np.ascontiguousarray(
            xc.transpose(2, 0, 1).reshape(2, 128, NSEQ * SEQLEN)
        ).astype(np.float16)
        in_maps.append({"xT": xT, "consts": cb, "constsh": ch})

    res = run_bass_kernel_spmd(nc, in_maps, core_ids=list(range(NCORES)))
    LAST_RUN_INFO.clear()
    LAST_RUN_INFO.update(
        exec_time_ns=res.exec_time_ns,
        mean_exec_time_ns=res.mean_exec_time_ns,
        trace=res.instructions_and_trace[1] if res.instructions_and_trace else None,
        profile_json=res.profile_json,
    )

    out = np.empty((BATCH, SEQLEN, DM), np.float32)
    for c in range(NCORES):
        oT = res.results[c]["outT"]                          # (2, 128, ntok)
        out[c * NSEQ:(c + 1) * NSEQ] = (
            oT.reshape(DM, NSEQ, SEQLEN).transpose(1, 2, 0)
        )
    return out


# revision 36
# speedup vs baseline: 1.0069x; 1.0007x over previous
"""LRU (Linear Recurrent Unit) block kernel for Trainium2, 8 NeuronCores.

Math (per batch element, see reference):
    lam  = exp(-exp(nu_log)) * exp(i*exp(theta_log))          (S,) complex
    Bn   = (B_re + i B_im) * exp(gamma_log)[:, None]          (S, D)
    Bu_t = Bn @ x_t                                           complex
    s_t  = lam * s_{t-1} + Bu_t                               diagonal complex scan
    z_t  = Re(C s_t) + D x_t
    out  = W_proj @ gelu(W_fc @ z + b_fc) + b_proj + x        (MLP + residual)

Device strategy (data-parallel over batch, 2 sequences/core):
  - Everything runs transposed: features on SBUF partitions, tokens on the
    free axis. x is pre-transposed on the host.
  - Complex scan via the modulus-phase decomposition: with lam = r*e^{i*th},
    v_tau = e^{-i*th*tau} s_tau obeys v_tau = r v_{tau-1} + e^{-i*th*tau} Bu_tau
    — TWO REAL first-order recurrences (DVE tensor_tensor_scan).
  - The twiddle (cos/sin modulation) runs in fp16 on the DVE in 2x mode;
    every element-wise op covers both 128-state halves at once via 4D APs
    (half the instruction count). Ops must NOT alias out with an input —
    in-place tensor_tensor silently drops to 1x mode. GpSimd is left idle
    on purpose: its software tensor ops contend for SBUF ports and slow
    the DVE ~2.5x while active.
  - PSUM is managed as one FIFO pool of four 2-bank (4KB) tiles; every
    scalar-engine PSUM drain (Bu copy, z copy, gelu, output bias) covers
    two banks per instruction, halving the scalar op count.
  - Phase pipeline per chunk: A(k+1) Bu-matmuls | C(k) MLP matmuls | B(k+1)
    twiddle+scan on the DVE, with the untwiddle split in halves so phase C
    can start on the first half-chunk early.
"""

import numpy as np

import concourse.bass as bass
import concourse.mybir as mybir
import concourse.tile as tile
from concourse.vector_clock import ScopedClock
from concourse.bass_utils import run_bass_kernel_spmd

Alu = mybir.AluOpType
F32 = mybir.dt.float32
F16 = mybir.dt.float16
ACTF = mybir.ActivationFunctionType
GELU_FUNC = ACTF.Gelu  # overridable for CoreSim (no Gelu in the interpreter)

BATCH, SEQLEN, DM, DS, DF = 16, 8192, 256, 256, 1024
NCORES = 8
NSEQ = BATCH // NCORES          # sequences per core
PC = 256                        # positions per PSUM sub-chunk (per sequence)
SCMAX = 1024                    # max positions per super-chunk
CHUNKS = (1024, 1024, 1024, 1024, 1024, 1024, 1024, 1024)
assert sum(CHUNKS) == SEQLEN

# ---- fp32 consts blob layout (columns of [128, NCOL]) ----------------------
RT0 = 0                         # scan decay r per st: 2 fp32 cols
ROT0 = RT0 + 2                  # carry rotation cos per (chunk, st), then sin
NCHUNK = len(CHUNKS)
BFC0 = ROT0 + 4 * NCHUNK        # fc1 bias per f-tile (8)
BPJ0 = BFC0 + 8                 # proj bias per o-tile (2)
NCOL = BPJ0 + 2

# ---- fp16 consts blob layout (columns of [128, NCOLH]) ---------------------
# 21 fp16 weight tiles: cr 4, cm 4, bnre 4, bnim 4, dT 4, identity 1
def _hi_cr(st, ot): return st * 2 + ot      # C_re^T tiles
def _hi_cm(st, ot): return 4 + st * 2 + ot  # -C_im^T tiles
def _hi_bnre(kt, st): return 8 + kt * 2 + st
def _hi_bnim(kt, st): return 12 + kt * 2 + st
def _hi_dT(kt, ot):   return 16 + kt * 2 + ot
_HI_IDENT = 20
def _hi_wfc(kt, ft):  return 21 + kt * 8 + ft
def _hi_wpj(ft, ot):  return 37 + ft * 2 + ot
CH0 = 53 * 128                  # cos table [st][tau]: 2*SCMAX cols
SH0 = CH0 + 2 * SCMAX           # sin table
NCOLH = SH0 + 2 * SCMAX


# --- tile-exit drain workaround: walrus in this container caps the sync-wait
# slots on a TPB_CTRL Drain; split the waits onto follow-up SP nops. ---------
def _patched_drain_and_barrier(self, tick_clock, wait_clock):
    nc = self.nc
    drain_inst = nc.sync.drain()
    wait_clock.add_sem_waits(
        drain_inst.ins, ScopedClock({None: tick_clock.global_clock})
    )
    si = drain_inst.ins.sync_info
    if si is not None and si.on_wait and len(si.on_wait) > 1:
        waits = list(si.on_wait)
        drain_inst.ins.sync_info = mybir.SyncInfo(
            on_wait=[waits[0]], on_update=list(si.on_update or [])
        )
        for w in waits[1:]:
            nop = nc.sync.nop(hint="drain_wait_split", nofuse=True)
            nop.ins.sync_info = mybir.SyncInfo(on_wait=[w], on_update=[])
    nc.all_engine_barrier()
    assert self.sems is not None
    popped = nc._tile_sem_poison_stack.pop()
    assert popped is self._sem_poison
    nc.clear_and_free_semaphores(list(self.sems.allocated().values()))
    nc.all_engine_barrier()


tile.TileContext._drain_and_barrier = _patched_drain_and_barrier


# --- universal sync-wait splitter: this walrus rejects >1 wait on several
# instruction structs (S3_LW matmul, TPB_CTRL drain, ...). Rewrite the
# serialized BIR so every instruction carries at most one wait; extras move
# to injected same-engine NoOps placed immediately before it. ----------------
def _split_sync_waits(bir: bytes) -> bytes:
    import json as _json

    m = _json.loads(bir)
    ctr = 0
    for f in m.get("functions", []):
        for bb in f.get("blocks", []):
            insts = bb.get("instructions")
            if not insts:
                continue
            out = []
            for inst in insts:
                si = inst.get("sync_info")
                ow = (si or {}).get("on_wait") or []
                if len(ow) > 1:
                    for wdesc in ow[:-1]:
                        ctr += 1
                        out.append({
                            "engine": inst["engine"],
                            "ins": [],
                            "outs": [],
                            "name": f"I-wsplit{ctr}",
                            "opcode": "NoOp",
                            "sync_info": {"on_update": [], "on_wait": [wdesc]},
                            "text_hint": "wait_split",
                        })
                    si["on_wait"] = [ow[-1]]
                out.append(inst)
            bb["instructions"] = out
    return _json.dumps(m).encode()


_orig_to_json_bytes = bass.Bass.to_json_bytes


def _to_json_bytes_split(self):
    return _split_sync_waits(_orig_to_json_bytes(self))


bass.Bass.to_json_bytes = _to_json_bytes_split


def _enable_axon_ntff_profiling():
    """Best-effort: register the axon NTFF profile hook (the image's antenv
    lacks axon_hooks; the backing ctypes impl ships in trn_agent_boot) and
    neuter the S3 artifact upload the trace path would attempt."""
    try:
        import sys, types
        try:
            import antenv.axon_hooks  # noqa: F401
        except ImportError:
            mod = types.ModuleType("antenv.axon_hooks")
            mod._hook = None

            def set_axon_ntff_profile_hook(h):
                mod._hook = h

            def get_axon_ntff_profile_hook():
                return mod._hook

            mod.set_axon_ntff_profile_hook = set_axon_ntff_profile_hook
            mod.get_axon_ntff_profile_hook = get_axon_ntff_profile_hook
            sys.modules["antenv.axon_hooks"] = mod
            import antenv
            antenv.axon_hooks = mod
        import antenv.axon_hooks as ah
        if ah.get_axon_ntff_profile_hook() is None:
            from trn_agent_boot.trn_boot import _ntff_profile_via_ctypes
            ah.set_axon_ntff_profile_hook(
                _ntff_profile_via_ctypes("/opt/axon/libaxon_pjrt.so")
            )
        import concourse.bass_utils as bu
        bu.upload_artifacts = lambda tmpdir: ""
    except Exception:
        pass


import os as _os
if _os.environ.get("BASS_TRACE"):
    _enable_axon_ntff_profiling()


def build_nc():
    """Per-core Bass module. Token layout: [nseq, seqlen] flattened."""
    ntok = NSEQ * SEQLEN

    nc = bass.Bass()
    xT = nc.declare_dram_parameter("xT", [2, 128, ntok], F16, isOutput=False)
    consts = nc.declare_dram_parameter("consts", [128, NCOL], F32, isOutput=False)
    constsh = nc.declare_dram_parameter("constsh", [128, NCOLH], F16, isOutput=False)
    outT = nc.declare_dram_parameter("outT", [2, 128, ntok], F32, isOutput=True)

    xTv = [xT[kt].rearrange("p (b l) -> p b l", b=NSEQ) for kt in range(2)]
    outTv = [outT[ot].rearrange("p (b l) -> p b l", b=NSEQ) for ot in range(2)]

    from contextlib import ExitStack
    with tile.TileContext(nc) as tc, ExitStack() as ctx:
        singles = ctx.enter_context(tc.tile_pool(name="singles", bufs=1))
        xts = ctx.enter_context(tc.tile_pool(name="xts", bufs=3))
        bus = ctx.enter_context(tc.tile_pool(name="bus", bufs=2))
        uts = ctx.enter_context(tc.tile_pool(name="uts", bufs=1))
        ss = ctx.enter_context(tc.tile_pool(name="ss", bufs=2))
        zs_p = ctx.enter_context(tc.tile_pool(name="zs", bufs=3))
        hs_p = ctx.enter_context(tc.tile_pool(name="hs", bufs=2))
        obs = ctx.enter_context(tc.tile_pool(name="obs", bufs=2))
        tmps = ctx.enter_context(tc.tile_pool(name="tmps", bufs=2))
        cartmps = ctx.enter_context(tc.tile_pool(name="cartmps", bufs=2))
        carries = ctx.enter_context(tc.tile_pool(name="carries", bufs=2))
        # paired-bank PSUM tiles: [128, 2(sub), NSEQ, PC] f32 = 4KB/part
        ps = ctx.enter_context(tc.tile_pool(name="ps", bufs=4, space="PSUM"))

        # consts DMA split by first consumer: Bn weights gate phase A(0),
        # cb (scan decay) + trig tables gate B(0), MLP weights gate C(0).
        cb = singles.tile([128, NCOL], F32, tag="consts")
        ch = singles.tile([128, NCOLH], F16, tag="constsh")
        nc.sync.dma_start(out=ch[:, 8 * 128:16 * 128],
                          in_=constsh[:, 8 * 128:16 * 128])
        nc.sync.dma_start(out=cb[:], in_=consts[:])
        nc.sync.dma_start(out=ch[:, CH0:], in_=constsh[:, CH0:])
        nc.sync.dma_start(out=ch[:, 0:8 * 128], in_=constsh[:, 0:8 * 128])
        nc.sync.dma_start(out=ch[:, 16 * 128:CH0],
                          in_=constsh[:, 16 * 128:CH0])

        def wh(i):  # fp16 weight tile i
            return ch[:, i * 128:(i + 1) * 128]

        def tab4(base, a, bnd):  # fp16 table [128, 2st, nseq, W] bcast on seq
            return ch[:, base: base + 2 * SCMAX] \
                .rearrange("p (s t) -> p s t", s=2)[:, :, None, a:bnd] \
                .to_broadcast([128, 2, NSEQ, bnd - a])

        # carry state [128, st, plane, b], fp32, zero-init
        carry = carries.tile([128, 2, 2, NSEQ], F32, tag="carry")
        nc.vector.memset(carry[:], 0.0)

        def phase_A(ci, lo, L):
            """Load x chunk, compute Bu into SBUF fp16 (via shared PSUM)."""
            subs = L // PC
            xt = []
            for kt in range(2):
                t = xts.tile([128, NSEQ, SCMAX], F16, tag=f"xt{kt}")
                nc.sync.dma_start(out=t[:, :, :L], in_=xTv[kt][:, :, lo:lo + L])
                xt.append(t)
            bu_re = bus.tile([128, 2, NSEQ, SCMAX], F16, tag="bure",
                             name="bure")
            bu_im = bus.tile([128, 2, NSEQ, SCMAX], F16, tag="buim",
                             name="buim")
            for s0 in range(0, subs, 2):
                g = min(2, subs - s0)
                for pl, bt in ((0, bu_re), (1, bu_im)):
                    for st in range(2):
                        psb = ps.tile([128, 2, NSEQ, PC], F32, tag="ps")
                        for gi in range(g):
                            o0 = (s0 + gi) * PC
                            for kt in range(2):
                                wi = (_hi_bnre(kt, st) if pl == 0
                                      else _hi_bnim(kt, st))
                                nc.tensor.matmul(
                                    psb[:, gi], wh(wi),
                                    xt[kt][:, :, o0:o0 + PC],
                                    start=(kt == 0), stop=(kt == 1))
                        bslice = bt[:, st, :, s0 * PC:(s0 + g) * PC] \
                            .rearrange("p b (g f) -> p g b f", g=g)
                        nc.scalar.activation(bslice, psb[:, :g], ACTF.Copy)
            return xt, (bu_re, bu_im)

        def phase_B(ci, L, bu_sb):
            """Twiddle -> scan -> untwiddle, all on the DVE. Every
            element-wise op covers BOTH state halves (4D APs) to halve the
            instruction count; op order minimizes the latency until phase
            C's first-half s tiles are available."""
            nonlocal carry
            bu_re, bu_im = bu_sb
            carry_new = carries.tile([128, 2, 2, NSEQ], F32, tag="carry")

            ur_t = uts.tile([128, 2, NSEQ, SCMAX], F16, tag="utre")
            ui_t = uts.tile([128, 2, NSEQ, SCMAX], F16, tag="utim")
            sr_t = ss.tile([128, 2, NSEQ, SCMAX], F16, tag="sre")
            si_t = ss.tile([128, 2, NSEQ, SCMAX], F16, tag="sim")

            def fwd(a, bnd):
                # forward twiddle (rotating frame): u = e^{-i*th*tau} * Bu
                cosb, sinb = tab4(CH0, a, bnd), tab4(SH0, a, bnd)
                W = bnd - a
                ur, ui = ur_t[:, :, :, a:bnd], ui_t[:, :, :, a:bnd]
                bre = bu_re[:, :, :, a:bnd]
                bim = bu_im[:, :, :, a:bnd]
                t1 = tmps.tile([128, 2, NSEQ, SCMAX], F16, tag="twtmp")
                t2 = tmps.tile([128, 2, NSEQ, SCMAX], F16, tag="twtmp")
                nc.vector.tensor_tensor(t1[:, :, :, :W], cosb, bre, Alu.mult)
                nc.vector.tensor_tensor(t2[:, :, :, :W], sinb, bim, Alu.mult)
                nc.vector.tensor_tensor(ur, t1[:, :, :, :W], t2[:, :, :, :W],
                                        Alu.add)
                t3 = tmps.tile([128, 2, NSEQ, SCMAX], F16, tag="twtmp")
                t4 = tmps.tile([128, 2, NSEQ, SCMAX], F16, tag="twtmp")
                nc.vector.tensor_tensor(t3[:, :, :, :W], cosb, bim, Alu.mult)
                nc.vector.tensor_tensor(t4[:, :, :, :W], sinb, bre, Alu.mult)
                nc.vector.tensor_tensor(ui, t3[:, :, :, :W], t4[:, :, :, :W],
                                        Alu.subtract)

            def scans(st, a, bnd):
                # scans run in place: v overwrites ut; chained at half
                # boundaries via initial = previous half's last column.
                # decay r is constant along tau: stride-0 broadcast column.
                rt2 = cb[:, RT0 + st: RT0 + st + 1].to_broadcast(
                    [128, bnd - a])
                for pl, t in ((0, ur_t), (1, ui_t)):
                    for b in range(NSEQ):
                        init = (carry[:, st, pl, b:b + 1] if a == 0
                                else t[:, st, b, a - 1:a])
                        nc.vector.tensor_tensor_scan(
                            t[:, st, b, a:bnd], rt2, t[:, st, b, a:bnd],
                            init, Alu.mult, Alu.add)

            def carry_upd():
                # carry for the next chunk: rotate by e^{+i*th*L}
                rotc = cb[:, ROT0 + ci * 2: ROT0 + ci * 2 + 2][:, :, None] \
                    .to_broadcast([128, 2, NSEQ])
                rots = cb[:, ROT0 + 2 * NCHUNK + ci * 2:
                          ROT0 + 2 * NCHUNK + ci * 2 + 2][:, :, None] \
                    .to_broadcast([128, 2, NSEQ])
                vrl = ur_t[:, :, :, L - 1]
                vil = ui_t[:, :, :, L - 1]
                ta = cartmps.tile([128, 2, NSEQ], F32, tag="cartmp")
                tb = cartmps.tile([128, 2, NSEQ], F32, tag="cartmp")
                nc.vector.tensor_tensor(ta[:], rots, vil, Alu.mult)
                nc.vector.tensor_tensor(tb[:], rotc, vrl, Alu.mult)
                nc.vector.tensor_tensor(carry_new[:, :, 0, :], tb[:], ta[:],
                                        Alu.subtract)
                tc = cartmps.tile([128, 2, NSEQ], F32, tag="cartmp")
                td = cartmps.tile([128, 2, NSEQ], F32, tag="cartmp")
                nc.vector.tensor_tensor(tc[:], rots, vrl, Alu.mult)
                nc.vector.tensor_tensor(td[:], rotc, vil, Alu.mult)
                nc.vector.tensor_tensor(carry_new[:, :, 1, :], td[:], tc[:],
                                        Alu.add)

            def untw(a, bnd):
                # untwiddle: s = e^{+i*th*tau} * v  (cols [a, bnd))
                cosb = tab4(CH0, a, bnd)
                sinb = tab4(SH0, a, bnd)
                vr = ur_t[:, :, :, a:bnd]
                vi = ui_t[:, :, :, a:bnd]
                sr = sr_t[:, :, :, a:bnd]
                si_ = si_t[:, :, :, a:bnd]
                u1 = tmps.tile([128, 2, NSEQ, SCMAX], F16, tag="twtmp")
                u2 = tmps.tile([128, 2, NSEQ, SCMAX], F16, tag="twtmp")
                W = bnd - a
                nc.vector.tensor_tensor(u1[:, :, :, :W], cosb, vr, Alu.mult)
                nc.vector.tensor_tensor(u2[:, :, :, :W], sinb, vi, Alu.mult)
                nc.vector.tensor_tensor(sr, u1[:, :, :, :W], u2[:, :, :, :W],
                                        Alu.subtract)
                u3 = tmps.tile([128, 2, NSEQ, SCMAX], F16, tag="twtmp")
                u4 = tmps.tile([128, 2, NSEQ, SCMAX], F16, tag="twtmp")
                nc.vector.tensor_tensor(u3[:, :, :, :W], cosb, vi, Alu.mult)
                nc.vector.tensor_tensor(u4[:, :, :, :W], sinb, vr, Alu.mult)
                nc.vector.tensor_tensor(si_, u3[:, :, :, :W], u4[:, :, :, :W],
                                        Alu.add)

            half = max(PC, (L + PC - 1) // (2 * PC) * PC)  # first-half cols
            fwd(0, half)
            scans(0, 0, half)
            scans(1, 0, half)
            if half < L:
                fwd(half, L)
                untw(0, half)
                scans(0, half, L)
                scans(1, half, L)
                carry_upd()
                untw(half, L)
            else:
                untw(0, half)
                carry_upd()

            carry = carry_new
            return sr_t, si_t

        def phase_C(ci, lo, L, xt, s_re, s_im):
            subs = L // PC
            for s0 in range(0, subs, 2):
                g = min(2, subs - s0)
                q0 = s0 * PC
                glo = lo + q0

                z_sb = []
                for ot in range(2):
                    zp = ps.tile([128, 2, NSEQ, PC], F32, tag="ps")
                    for gi in range(g):
                        o0, o1 = q0 + gi * PC, q0 + (gi + 1) * PC
                        zg = zp[:, gi]
                        nc.tensor.matmul(zg, wh(_hi_dT(0, ot)),
                                         xt[0][:, :, o0:o1],
                                         start=True, stop=False)
                        nc.tensor.matmul(zg, wh(_hi_dT(1, ot)),
                                         xt[1][:, :, o0:o1],
                                         start=False, stop=False)
                        nc.tensor.matmul(zg, wh(_hi_cr(0, ot)),
                                         s_re[:, 0, :, o0:o1],
                                         start=False, stop=False)
                        nc.tensor.matmul(zg, wh(_hi_cm(0, ot)),
                                         s_im[:, 0, :, o0:o1],
                                         start=False, stop=False)
                        nc.tensor.matmul(zg, wh(_hi_cr(1, ot)),
                                         s_re[:, 1, :, o0:o1],
                                         start=False, stop=False)
                        nc.tensor.matmul(zg, wh(_hi_cm(1, ot)),
                                         s_im[:, 1, :, o0:o1],
                                         start=False, stop=True)
                    zt = zs_p.tile([128, 2, NSEQ, PC], F16, tag=f"z{ot}")
                    nc.scalar.activation(zt[:, :g], zp[:, :g], ACTF.Copy)
                    z_sb.append(zt)

                h_sb = []
                for ft in range(8):
                    hp = ps.tile([128, 2, NSEQ, PC], F32, tag="ps")
                    for gi in range(g):
                        nc.tensor.matmul(hp[:, gi], wh(_hi_wfc(0, ft)),
                                         z_sb[0][:, gi], start=True, stop=False)
                        nc.tensor.matmul(hp[:, gi], wh(_hi_wfc(1, ft)),
                                         z_sb[1][:, gi], start=False, stop=True)
                    ht = hs_p.tile([128, 2, NSEQ, PC], F16, tag=f"h{ft}")
                    nc.scalar.activation(
                        ht[:, :g], hp[:, :g], GELU_FUNC,
                        bias=cb[:, BFC0 + ft: BFC0 + ft + 1],
                        scale=1.0)
                    h_sb.append(ht)

                for ot in range(2):
                    qp = ps.tile([128, 2, NSEQ, PC], F32, tag="ps")
                    for gi in range(g):
                        o0, o1 = q0 + gi * PC, q0 + (gi + 1) * PC
                        nc.tensor.matmul(qp[:, gi], wh(_HI_IDENT),
                                         xt[ot][:, :, o0:o1],
                                         start=True, stop=False)
                        for ft in range(8):
                            nc.tensor.matmul(qp[:, gi], wh(_hi_wpj(ft, ot)),
                                             h_sb[ft][:, gi],
                                             start=False, stop=(ft == 7))
                    ob = obs.tile([128, 2, NSEQ, PC], F32, tag=f"ob{ot}")
                    nc.scalar.activation(
                        ob[:, :g], qp[:, :g], ACTF.Identity,
                        bias=cb[:, BPJ0 + ot: BPJ0 + ot + 1],
                        scale=1.0)
                    for gi in range(g):
                        nc.sync.dma_start(
                            out=outTv[ot][:, :, glo + gi * PC:
                                          glo + (gi + 1) * PC],
                            in_=ob[:, gi])

        # software pipeline: A(c) | C(c-1) | B(c), so the PE stream
        # always has ready work while the DVE/GpSimd run phase B.
        prev = None
        lo = 0
        for ci, L in enumerate(CHUNKS):
            xt, bu_sb = phase_A(ci, lo, L)
            if prev is not None:
                phase_C(*prev)
            s_re, s_im = phase_B(ci, L, bu_sb)
            prev = (ci, lo, L, xt, s_re, s_im)
            lo += L
        phase_C(*prev)
    return nc


def pack_consts(nu_log, theta_log, gamma_log, B_re, B_im, C_re, C_im, D,
                W_fc, b_fc, W_proj, b_proj):
    """Assemble the f32 and fp16 consts blobs (tables in float64)."""
    f8 = np.float64
    nu = np.exp(np.asarray(nu_log, f8))
    r = np.exp(-nu)
    theta = np.exp(np.asarray(theta_log, f8))
    gamma = np.exp(np.asarray(gamma_log, f8))
    Bn_re = np.asarray(B_re, f8) * gamma[:, None]
    Bn_im = np.asarray(B_im, f8) * gamma[:, None]
    C_re = np.asarray(C_re, f8)
    C_im = np.asarray(C_im, f8)
    D = np.asarray(D, f8)
    W_fc = np.asarray(W_fc, f8)
    W_proj = np.asarray(W_proj, f8)

    cb = np.zeros((128, NCOL), np.float32)
    ch = np.zeros((128, NCOLH), np.float16)

    def puth(i, m):
        ch[:, i * 128:(i + 1) * 128] = np.asarray(m, np.float16)

    for kt in range(2):
        for st in range(2):
            puth(_hi_bnre(kt, st),
                 Bn_re[st * 128:(st + 1) * 128, kt * 128:(kt + 1) * 128].T)
            puth(_hi_bnim(kt, st),
                 Bn_im[st * 128:(st + 1) * 128, kt * 128:(kt + 1) * 128].T)
    for st in range(2):
        for ot in range(2):
            puth(_hi_cr(st, ot),
                 C_re[ot * 128:(ot + 1) * 128, st * 128:(st + 1) * 128].T)
            puth(_hi_cm(st, ot),
                 -C_im[ot * 128:(ot + 1) * 128, st * 128:(st + 1) * 128].T)
    for kt in range(2):
        for ot in range(2):
            puth(_hi_dT(kt, ot),
                 D[ot * 128:(ot + 1) * 128, kt * 128:(kt + 1) * 128].T)
    for kt in range(2):
        for ft in range(8):
            puth(_hi_wfc(kt, ft),
                 W_fc[kt * 128:(kt + 1) * 128, ft * 128:(ft + 1) * 128])
    for ft in range(8):
        for ot in range(2):
            puth(_hi_wpj(ft, ot),
                 W_proj[ft * 128:(ft + 1) * 128, ot * 128:(ot + 1) * 128])
    puth(_HI_IDENT, np.eye(128))

    tau = np.arange(SCMAX, dtype=f8)
    for st in range(2):
        th = theta[st * 128:(st + 1) * 128]
        ang = th[:, None] * tau[None, :]
        ch[:, CH0 + st * SCMAX: CH0 + (st + 1) * SCMAX] = np.cos(ang)
        ch[:, SH0 + st * SCMAX: SH0 + (st + 1) * SCMAX] = np.sin(ang)
        cb[:, RT0 + st] = r[st * 128:(st + 1) * 128]
        for ci, L in enumerate(CHUNKS):
            cb[:, ROT0 + ci * 2 + st] = np.cos(th * L)
            cb[:, ROT0 + 2 * NCHUNK + ci * 2 + st] = np.sin(th * L)
    for ft in range(8):
        cb[:, BFC0 + ft] = np.asarray(b_fc, np.float32)[ft * 128:(ft + 1) * 128]
    for ot in range(2):
        cb[:, BPJ0 + ot] = np.asarray(b_proj, np.float32)[ot * 128:(ot + 1) * 128]
    return cb, ch


_NC_CACHE = {}
LAST_RUN_INFO = {}


def kernel(x, nu_log, theta_log, gamma_log, B_re, B_im, C_re, C_im, D,
           W_fc, b_fc, W_proj, b_proj):
    x = np.asarray(x, np.float32)
    assert x.shape == (BATCH, SEQLEN, DM)

    key = (SEQLEN, NSEQ, PC, CHUNKS)
    if key not in _NC_CACHE:
        _NC_CACHE[key] = build_nc()
    nc = _NC_CACHE[key]

    cb, ch = pack_consts(nu_log, theta_log, gamma_log, B_re, B_im, C_re, C_im,
                         D, W_fc, b_fc, W_proj, b_proj)

    in_maps = []
    for c in range(NCORES):
        xc = x[c * NSEQ:(c + 1) * NSEQ]                      # (nseq, L, D)
        xT = np.ascontiguousarray(
            xc.transpose(2, 0, 1).reshape(2, 128, NSEQ * SEQLEN)
        ).astype(np.float16)
        in_maps.append({"xT": xT, "consts": cb, "constsh": ch})

    res = run_bass_kernel_spmd(nc, in_maps, core_ids=list(range(NCORES)))
    LAST_RUN_INFO.clear()
    LAST_RUN_INFO.update(
        exec_time_ns=res.exec_time_ns,
        mean_exec_time_ns=res.mean_exec_time_ns,
        trace=res.instructions_and_trace[1] if res.instructions_and_trace else None,
        profile_json=res.profile_json,
    )

    out = np.empty((BATCH, SEQLEN, DM), np.float32)
    for c in range(NCORES):
        oT = res.results[c]["outT"]                          # (2, 128, ntok)
        out[c * NSEQ:(c + 1) * NSEQ] = (
            oT.reshape(DM, NSEQ, SEQLEN).transpose(1, 2, 0)
        )
    return out
